# revision 37
# baseline (speedup 1.0000x reference)
"""Causal multi-head attention (B=1, S=4096, H=16 heads x 64, hidden 1024) on
8 Trainium2 NeuronCores.

Sharding: tensor-parallel over heads, 2 heads per core. Each core receives the
full activation (pre-transposed to [hidden, S] layout), its 128-row slice of
wq/wk/wv (transposed) and 128-column slice of wo (transposed), computes
q/k/v projections + flash-style causal attention for its 2 heads, applies its
slice of the output projection, and writes a full-shape partial output (fp16).
The host sums the 8 partials (the TP all-reduce) to produce the final output.

Kernel layout notes (v3):
  - scores are computed TRANSPOSED: ST[sk, sq] = kT_tile^T @ qT_block, so the
    softmax numerator exp() runs PSUM->SBUF on the scalar engine with no
    transposes of the probability matrix anywhere.
  - a slot = ONE 128-row k-tile x BOTH heads. The two heads' score matmuls
    have K=64 (head dim) and disjoint stationary partition ranges (0-63 /
    64-127), so bass auto-derives row_grp=h0/h64 tile positions and the PE
    runs the pair CONCURRENTLY in the two halves of the array (row tiling).
    This halves the PE time of the score phase vs. sequential K=64 matmuls.
  - both heads share ONE exp() instruction per slot: h0's valid columns land
    at [off, 512), h64's are packed at [512, 1024-off), so the valid region
    is contiguous and one activation covers it. The scalar engine is the
    bottleneck engine (exp over the whole causal triangle is ~110us of pure
    element throughput), so no wasted exp columns and as few instruction
    access-latency taxes as possible.
  - the softmax denominator comes for free from the PV matmul by augmenting
    v with a ones column (stationary operand [v | 1], M=65): output row 64
    accumulates sum_k exp(s).
  - per-head l rows are broadcast across the head's 64 partitions with a
    pair of col-tiled (M=64) PE matmuls (concurrent), and 1/l = exp(-ln l)
    runs on the scalar engine. (The DVE reciprocal is a ~3.4us multi-pass
    instruction that stalls the FIFO vector queue and with it the whole
    block boundary — measured; Ln/Exp slot into the exp stream instead.)
  - matmul operands are float16 (11-bit mantissa ~ TF32 accuracy): 2-byte
    weight loads pipeline behind the matmul stream and keep the PE's HAM
    clock gate warm (4-byte fp32/fp32r weight loads serialize, halving
    effective clock). PSUM accumulation is fp32 throughout. Output partials
    are stored fp16 (halves the output DMA) and summed fp32 on the host.
  - emission is a software-pipelined work queue with per-piece deadlines:
    each attention slot emits the row-tiled score pair, exp, mask, and a
    4-slot-lagged PV pair, plus one drip-fed piece of deferred work (the
    previous block's normalize/out-proj, the next block's projections).
    Pieces whose deadline allows it carry across block boundaries so the
    boundary never runs a serial flush while the scalar engine idles.
    Keeping the PE dense also keeps the HAM clock gate at 8/8 (2.4 GHz).
  - weights/x use a (p k) contraction-chunk layout so every weight-load DMA
    descriptor is a contiguous 2KB line (the (k p) layout's 256B descriptors
    made the startup weight loads descriptor-bound). A burst of tiny warm-up
    matmuls on a memset tile trips the HAM activity window during the
    startup DMA wait so the first projections run at 2.4 GHz.
  - the last block's normalize/out-proj is emitted per 128-column chunk,
    pinned right after the PV that finalizes that chunk, so the kernel tail
    is a staggered pipeline instead of a serial norm -> 8 matmuls -> stores
    chain.
"""
import sys
sys.path.insert(0, "/opt/trn_rl_repo")

import numpy as np

import concourse.bass as bass
import concourse.mybir as mybir
import concourse.tile as tile
from concourse.bass_utils import run_bass_kernel_spmd

# ---------------------------------------------------------------- constants
S = 4096          # sequence length
HID = 1024        # hidden dim
NCORES = 8
HPC = 2           # heads per core
HD = 64           # head dim
EPC = HPC * HD    # 128 e-dims (head-concat) per core
SB = 512          # q-block width
NB = S // SB      # 8 q-blocks
NT = S // 128     # 32 k-tiles

F32 = mybir.dt.float32
F16 = mybir.dt.float16
DT = F16  # matmul operand dtype

_MAX_WAITS = 1    # this walrus build allows a single sync-wait per instruction


def _split_waits(nc):
    """Hoist extra sync-waits onto inserted same-engine drain carriers."""
    n = 0
    for fn in nc.m.functions:
        for bb in fn.blocks:
            insts = bb.instructions
            i = 0
            while i < len(insts):
                inst = insts[i]
                si = inst.sync_info
                w = list(si.on_wait) if si is not None and si.on_wait else []
                if len(w) > _MAX_WAITS:
                    chunks = [w[j:j + _MAX_WAITS] for j in range(0, len(w), _MAX_WAITS)]
                    si.on_wait = chunks[-1]
                    for ch in chunks[:-1]:
                        # EventSemaphore carrier: pure wait, no pipeline flush
                        # (InstDrain stalls the engine pipe ~1.5us per use).
                        d = mybir.InstEventSemaphore(
                            name=f"{inst.name}_ws{n}", ins=[], outs=[])
                        d.engine = inst.engine
                        d.sync_info = mybir.SyncInfo(on_wait=ch, on_update=[])
                        insts.insert(i, d)
                        i += 1
                        n += 1
                i += 1
    return n


def _build_nc():
    nc = bass.Bass(target_bir_lowering=False)

    xT = nc.declare_dram_parameter("xT", [HID, S], DT, isOutput=False)
    wqT = nc.declare_dram_parameter("wqT", [HID, EPC], DT, isOutput=False)
    wkT = nc.declare_dram_parameter("wkT", [HID, EPC], DT, isOutput=False)
    wvT = nc.declare_dram_parameter("wvT", [HID, EPC], DT, isOutput=False)
    woT = nc.declare_dram_parameter("woT", [EPC, HID], DT, isOutput=False)
    cmask = nc.declare_dram_parameter("cmask", [128, 128], DT, isOutput=False)
    ones = nc.declare_dram_parameter("ones", [1, 128], DT, isOutput=False)
    ident = nc.declare_dram_parameter("ident", [128, 128], DT, isOutput=False)
    out = nc.declare_dram_parameter("out", [S, HID], DT, isOutput=True)

    KH = HID // 128  # 8 contraction chunks for projections

    with tile.TileContext(nc) as tc:
        with tc.tile_pool(name="const", bufs=1) as const, \
             tc.tile_pool(name="qk", bufs=1) as qk, \
             tc.tile_pool(name="xt", bufs=3) as xtp, \
             tc.tile_pool(name="vt", bufs=2) as vtp, \
             tc.tile_pool(name="pt", bufs=5) as ptp, \
             tc.tile_pool(name="att", bufs=2) as attp, \
             tc.tile_pool(name="osb", bufs=4) as osbp, \
             tc.tile_pool(name="rl", bufs=4) as rlp, \
             tc.tile_pool(name="bc", bufs=2) as bcp, \
             tc.tile_pool(name="st", bufs=2, space="PSUM") as stp, \
             tc.tile_pool(name="sc", bufs=2, space="PSUM") as scp, \
             tc.tile_pool(name="ot", bufs=2, space="PSUM") as otp:

            wq_sb = const.tile([128, KH, EPC], DT, tag="wq")
            wk_sb = const.tile([128, KH, EPC], DT, tag="wk")
            wv_sb = const.tile([128, KH, EPC], DT, tag="wv")
            wo_sb = const.tile([EPC, HID], DT, tag="wo")
            cm_sb = const.tile([128, 128], DT, tag="cm")
            ones_sb = const.tile([1, 128], DT, tag="ones")
            id_sb = const.tile([128, 128], DT, tag="id")
            wtile = const.tile([128, 512], DT, tag="warm")
            wdump = const.tile([64, 1], DT, tag="wdump")
            qT = qk.tile([128, S], DT, tag="qT")   # [e(2 heads), s]
            kT = qk.tile([128, S], DT, tag="kT")
            vbuf = qk.tile([128, HPC, NT, 65], DT, tag="v")  # [sk, h, t, v|1]

            def warmup_mms():
                # trip the HAM activity window during the startup DMA wait so
                # the first projections run at 2.4 GHz instead of 1.2
                nc.gpsimd.memset(wtile, 0.25)
                wps = scp.tile([128, 512], F32, tag="sc", name="warmps")
                # just enough to bridge to the first projections (which then
                # keep HAM busy themselves) — a longer burst delays them
                for _ in range(6):
                    nc.tensor.matmul(wps[0:64, :], wtile[:, 0:64],
                                     wtile, start=True, stop=True)
                return wps

            def load_consts():
                # weights spread across independent DMA queues; (p k) layout
                # = one contiguous 2KB descriptor per partition
                # all weights on the scalar HWDGE queue in need-order, only
                # tiny constants on gpsimd: active queues share the 16 DMA
                # engines, so every extra busy queue cuts the critical path's
                # share (sync carries xt only)
                nc.scalar.dma_start(
                    out=wq_sb, in_=wqT.rearrange("(p k) m -> p k m", p=128))
                nc.scalar.dma_start(
                    out=wk_sb, in_=wkT.rearrange("(p k) m -> p k m", p=128))
                nc.scalar.dma_start(
                    out=wv_sb, in_=wvT.rearrange("(p k) m -> p k m", p=128))
                nc.scalar.dma_start(out=wo_sb, in_=woT[:, :])
                nc.gpsimd.dma_start(out=id_sb, in_=ident[:, :])
                nc.gpsimd.dma_start(out=cm_sb, in_=cmask[:, :])
                nc.gpsimd.dma_start(out=ones_sb, in_=ones[:, :])
                nc.vector.memset(
                    vbuf.rearrange("p a b c -> p (a b c)"), 1.0)

            def load_xt(bb):
                xt = xtp.tile([128, KH, SB], DT, tag="xt", name=f"xt{bb}")
                src_ap = xT.rearrange("(p k) s -> p k s", p=128)
                if bb == 0:
                    # split the first load so the first projection matmuls
                    # (k-chunks 0..3) can start after half the transfer
                    nc.sync.dma_start(out=xt[:, 0:KH // 2, :],
                                      in_=src_ap[:, 0:KH // 2, 0:SB])
                    nc.sync.dma_start(out=xt[:, KH // 2:, :],
                                      in_=src_ap[:, KH // 2:, 0:SB])
                else:
                    nc.sync.dma_start(
                        out=xt, in_=src_ap[:, :, bb * SB:(bb + 1) * SB])
                return xt

            def norm_chunk(b, ots, m, csl):
                """Normalize att columns [m*128,(m+1)*128) of block b.
                csl = slice(m*128, (m+1)*128). Allocations are lazy (inside
                the closure) so the scratch-pool rotation follows run order."""
                holder = nrm_state[b]
                if m == 0:
                    holder["att"] = attp.tile([128, SB], DT, tag="att",
                                              name=f"att{b}")
                att = holder["att"]
                w = csl.stop - csl.start
                bcps = scp.tile([128, 512], F32, tag="sc", name=f"bc{b}_{m}")
                t1 = bcp.tile([128, 512], F32, tag="bc", name=f"ln{b}_{m}")
                lrows = []
                for h in range(HPC):
                    lrow = rlp.tile([1, SB], DT, tag="rl", name=f"rl{b}_{h}_{m}")
                    nc.vector.tensor_copy(out=lrow[:, 0:w],
                                          in_=ots[h][64:65, csl])
                    lrows.append(lrow)
                # col-tiled pair: out base partitions 0/64 -> concurrent
                for h in range(HPC):
                    nc.tensor.matmul(bcps[64 * h:64 * (h + 1), 0:w],
                                     ones_sb[:, 0:64], lrows[h][:, 0:w],
                                     start=True, stop=True)
                # 1/l = exp(-ln l) on the scalar engine
                nc.scalar.activation(out=t1[:, 0:w], in_=bcps[:, 0:w],
                                     func=mybir.ActivationFunctionType.Ln)
                nc.scalar.activation(out=t1[:, 0:w], in_=t1[:, 0:w],
                                     func=mybir.ActivationFunctionType.Exp,
                                     scale=-1.0)
                for h in range(HPC):
                    nc.vector.tensor_mul(att[64 * h:64 * (h + 1), csl],
                                         ots[h][0:64, csl],
                                         t1[64 * h:64 * (h + 1), 0:w])

            def outproj(b, mlist, tail=False):
                att = nrm_state[b]["att"]
                for m in mlist:
                    osb = osbp.tile([128, HID], DT, tag="osb",
                                    name=f"osb{b}_{m}")
                    for n2 in range(2):
                        op = scp.tile([128, 512], F32, tag="sc",
                                      name=f"op{b}_{m}_{n2}")
                        nc.tensor.matmul(op, att[:, m * 128:(m + 1) * 128],
                                         wo_sb[:, n2 * 512:(n2 + 1) * 512],
                                         start=True, stop=True)
                        if tail and n2 == 1:
                            # at the kernel tail the scalar engine is idle:
                            # split the evictions across DVE and scalar so
                            # the PSUM->SBUF chain isn't DVE-serial
                            nc.scalar.activation(
                                out=osb[:, n2 * 512:(n2 + 1) * 512], in_=op,
                                func=mybir.ActivationFunctionType.Copy)
                        else:
                            nc.vector.tensor_copy(
                                out=osb[:, n2 * 512:(n2 + 1) * 512], in_=op)
                    r0 = (4 * b + m) * 128
                    # stores ride the (otherwise idle) GpSimd SWDGE queues
                    nc.gpsimd.dma_start(out=out[r0:r0 + 128, :], in_=osb)

            def make_norm_pieces(b, ots):
                """Block b's normalize+out-proj as drip pieces: [bcast+recip
                (whole block), op0, op1, op2, op3]."""
                def norm_all():
                    norm_chunk(b, ots, 0, slice(0, SB))
                return [norm_all] + \
                    [lambda m=m: outproj(b, [m]) for m in range(4)]

            def make_proj_chunks(bb):
                """Projection work for block bb as slot-sized pieces (each
                ~2-4 matmuls of PE work, so a drip-fed piece never delays the
                next slot's score matmuls for long)."""
                slb = slice(bb * SB, (bb + 1) * SB)
                holder = {}

                def c_w(w_sb, dst, half):
                    def run():
                        if half == 0:
                            ps = scp.tile([128, SB], F32, tag="sc",
                                          name=f"ps{bb}_{dst.name}")
                            holder[dst.name] = ps
                        else:
                            ps = holder[dst.name]
                        for k in range(KH // 2 * half, KH // 2 * (half + 1)):
                            nc.tensor.matmul(ps, w_sb[:, k, :], xts[bb][:, k, :],
                                             start=(k == 0), stop=(k == KH - 1))
                        if half == 1:
                            nc.vector.tensor_copy(out=dst[:, slb], in_=ps)
                    return run

                def c_v(half):
                    def run():
                        if half == 0:
                            ps = scp.tile([128, SB], F32, tag="sc",
                                          name=f"psv{bb}")
                            holder["psv"] = ps
                        else:
                            ps = holder["psv"]
                        for k in range(KH // 2 * half, KH // 2 * (half + 1)):
                            nc.tensor.matmul(ps, wv_sb[:, k, :], xts[bb][:, k, :],
                                             start=(k == 0), stop=(k == KH - 1))
                        if half == 1:
                            vt = vtp.tile([128, SB], DT, tag="vt",
                                          name=f"vt{bb}")
                            nc.vector.tensor_copy(out=vt, in_=ps)
                            holder["vt"] = vt
                    return run

                def c_flips(jj):
                    def run():
                        vt = holder["vt"]
                        for j in jj:
                            t = 4 * bb + j
                            fp = scp.tile([128, 128], F32, tag="sc",
                                          name=f"fp{bb}_{j}")
                            nc.tensor.matmul(fp, vt[:, j * 128:(j + 1) * 128],
                                             id_sb, start=True, stop=True)
                            nc.vector.tensor_copy(out=vbuf[:, 0, t, 0:64],
                                                  in_=fp[:, 0:64])
                            nc.vector.tensor_copy(out=vbuf[:, 1, t, 0:64],
                                                  in_=fp[:, 64:128])
                    return run

                return [c_w(wq_sb, qT, 0), c_w(wq_sb, qT, 1),
                        c_w(wk_sb, kT, 0), c_w(wk_sb, kT, 1),
                        c_v(0), c_v(1),
                        c_flips([0, 1]), c_flips([2, 3])]

            xts = {}
            nrm_state = {b: {} for b in range(NB)}

            wps = warmup_mms()
            xts[0] = load_xt(0)      # xt(0) DMA first on the sync queue
            load_consts()            # (vbuf memset heads the DVE queue)
            xts[1] = load_xt(1)      # behind xt(0), ready by block 0's drip
            # consume the warm-up tile so the scratch pool sees it freed
            nc.vector.tensor_copy(out=wdump, in_=wps[0:64, 0:1])
            for c in make_proj_chunks(0):   # bootstrap block 0 projections
                c()

            pending = []    # (deadline_block, closure) work queue
            prev_no = []    # norm/out-proj pieces of the previous block
            for b in range(NB):
                pjn = make_proj_chunks(b + 1) if b + 1 < NB else None
                no = prev_no
                # queue: norm first (PV of this block waits on the previous
                # block's ot buffers), then xt prefetch + q-projection (hard
                # deadline: next block's first slot), then out-projections
                # interleaved between COMPLETE projection half-pairs (the
                # 2-buffer PSUM scratch rotation requires a tile's eviction
                # emitted before the tile allocated 2 later is written).
                adds = []
                if no:
                    adds.append((b, no[0]))          # bcast + recip + muls
                if b + 2 < NB:
                    def mk_pref(bb=b + 2):
                        def run():
                            xts[bb] = load_xt(bb)
                        return run
                    adds.append((b, mk_pref()))      # xt prefetch (2 ahead)
                head = adds
                mid = []
                if pjn:
                    mid += [(b, pjn[0]), (b, pjn[1])]    # q-proj halves
                inter = []
                ops = list(no[1:]) if no else []
                pairs = [pjn[2:4], pjn[4:6], pjn[6:8]] if pjn else [[], [], []]
                if ops:
                    inter.append((b + 1, ops.pop(0)))
                for pr in pairs:
                    inter += [(b + 1, c) for c in pr]
                    if ops:
                        inter.append((b + 1, ops.pop(0)))
                inter += [(b + 1, c) for c in ops]
                pending = head + pending + mid + inter

                # ---------- attention for q-block b (both heads per slot)
                ntl = 4 * (b + 1)  # causal k-tiles
                ots = [otp.tile([65, SB], F32, tag="ot", name=f"ot{b}_{h}")
                       for h in range(HPC)]
                pend = []   # pending (t, off, pt) awaiting PV emission
                tail_sp = (None, 0)   # last diag st tile's spare columns

                def emit_pv(t, off, pt):
                    # h0 valid pt cols [off, 512); h64 packed [512, 1024-off)
                    nc.tensor.matmul(
                        ots[0][:, off:], vbuf[:, 0, t, :], pt[:, off:SB],
                        start=(t == 0), stop=(t == ntl - 1),
                        skip_group_check=True)
                    nc.tensor.matmul(
                        ots[1][:, off:], vbuf[:, 1, t, :],
                        pt[:, SB:2 * SB - off],
                        start=(t == 0), stop=(t == ntl - 1),
                        skip_group_check=True)

                last = b == NB - 1
                # pace the deferred-piece drip over slots [0, ntl-4): the
                # last slots stay drip-free so the PE carries no backlog
                # into the next block's score matmuls (measured: a late
                # dripped projection delays them ~3us and the scalar engine
                # idles the whole time)
                Q = len(pending)
                W = max(1, ntl - 4)
                emitted = 0
                for t in range(ntl):
                    off = 128 * (t - 4 * b) if t >= 4 * b else 0
                    st = stp.tile([128, 2 * SB], F32, tag="st",
                                  name=f"st{b}_{t}")
                    # the two heads' K=64 score matmuls: disjoint stationary
                    # partition ranges -> row-tiled, run concurrently
                    nc.tensor.matmul(
                        st[:, off:SB],
                        kT[0:64, t * 128:(t + 1) * 128],
                        qT[0:64, b * SB + off:(b + 1) * SB],
                        start=True, stop=True)
                    nc.tensor.matmul(
                        st[:, SB:2 * SB - off],
                        kT[64:128, t * 128:(t + 1) * 128],
                        qT[64:128, b * SB + off:(b + 1) * SB],
                        start=True, stop=True)
                    if last and off:
                        # the last block's diagonal slots are small; keep
                        # the HAM activity window busy with a warm-up matmul
                        # into the st tile's unused spare columns, else the
                        # whole tail (PVs, out-proj, norms) runs at 1.2 GHz
                        tail_sp = (st, off)
                        nc.tensor.matmul(st[0:64, 2 * SB - off:],
                                         wtile[:, 0:64], wtile[:, 0:off],
                                         start=True, stop=True)
                    pt = ptp.tile([128, 2 * SB], DT, tag="pt",
                                  name=f"pt{b}_{t}")
                    nc.scalar.activation(out=pt[:, off:2 * SB - off],
                                         in_=st[:, off:2 * SB - off],
                                         func=mybir.ActivationFunctionType.Exp,
                                         scale=float(HD) ** -0.5)
                    if t >= 4 * b:  # triangle mask on the 128-wide diagonal
                        nc.vector.tensor_mul(pt[:, off:off + 128],
                                             pt[:, off:off + 128],
                                             cm_sb[:, 0:128])
                        nc.vector.tensor_mul(pt[:, SB:SB + 128],
                                             pt[:, SB:SB + 128],
                                             cm_sb[:, 0:128])
                    pend.append((t, off, pt))
                    # drip BEFORE the PV pops: a dripped piece may produce
                    # the vbuf tiles the popped PV consumes (same-slot case),
                    # and the PE queue is FIFO — the producer must be emitted
                    # first or the queue deadlocks
                    if t < W:
                        target = ((t + 1) * Q + W - 1) // W
                        while emitted < min(target, emitted + 2) and pending:
                            pending.pop(0)[1]()
                            emitted += 1
                    # PV lags 4 slots (hides the block-start ot/norm chain),
                    # shrinking near the block end so ot columns finalize
                    # early enough to pipeline the tail
                    lag = 4 if t < ntl - 7 else (2 if t < ntl - 3 else 1)
                    while len(pend) > lag:
                        emit_pv(*pend.pop(0))
                    if last and t >= ntl - 3:
                        # tail pipeline: chunk m's ot columns are final once
                        # PV(ntl-4+m) has run (popped at slot ntl-3+m).
                        # Out-projections wait until every PV is emitted —
                        # an out-proj matmul ahead of a PV in the PE FIFO
                        # serializes the whole tail (measured 4.3us/chunk).
                        m = t - (ntl - 3)
                        norm_chunk(b, ots, m, slice(m * 128, (m + 1) * 128))
                        if tail_sp[0] is not None:
                            tst, toff = tail_sp
                            nc.tensor.matmul(tst[0:64, 2 * SB - toff:],
                                             wtile[:, 0:64],
                                             wtile[:, 0:toff],
                                             start=True, stop=True)
                def tail_warm(n):
                    if last and tail_sp[0] is not None:
                        tst, toff = tail_sp
                        for _ in range(n):
                            nc.tensor.matmul(tst[0:64, 2 * SB - toff:],
                                             wtile[:, 0:64], wtile[:, 0:toff],
                                             start=True, stop=True)

                while pend:
                    t0 = pend[0][0]
                    emit_pv(*pend.pop(0))
                    tail_warm(2)
                    if last and t0 >= ntl - 4:
                        m = t0 - (ntl - 4)
                        norm_chunk(b, ots, m, slice(m * 128, (m + 1) * 128))
                if last:
                    for m in range(4):
                        outproj(b, [m], tail=True)
                        tail_warm(3)
                # flush pieces whose deadline is this block; carry the rest
                flush = [c for dl, c in pending if dl <= b]
                pending = [(dl, c) for dl, c in pending if dl > b]
                for c in flush:
                    c()
                prev_no = make_norm_pieces(b, ots) if not last else []

            for dl, c in pending:
                c()

    _split_waits(nc)
    return nc


_cached = {}


def _get_nc():
    if "nc" not in _cached:
        _cached["nc"] = _build_nc()
    return _cached["nc"]


def make_in_maps(x, wq, wk, wv, wo):
    x = np.asarray(x, dtype=np.float32)
    wq, wk, wv, wo = (np.asarray(a, dtype=np.float32) for a in (wq, wk, wv, wo))
    B = x.shape[0]
    assert x.shape == (B, S, HID)

    dt = np.float16
    xT = np.ascontiguousarray(x[0].T.astype(dt))            # [HID, S]
    # static causal mask for the 128-wide diagonal blocks
    p = np.arange(128)[:, None]
    i = np.arange(128)[None, :]
    cm = (p <= i).astype(dt)                                # [128, 128]
    ones = np.ones((1, 128), dtype=dt)
    ident = np.eye(128, dtype=dt)

    in_maps = []
    for c in range(NCORES):
        esl = slice(c * EPC, (c + 1) * EPC)
        in_maps.append({
            "xT": xT,
            "wqT": np.ascontiguousarray(wq[esl, :].T.astype(dt)),
            "wkT": np.ascontiguousarray(wk[esl, :].T.astype(dt)),
            "wvT": np.ascontiguousarray(wv[esl, :].T.astype(dt)),
            "woT": np.ascontiguousarray(wo[:, esl].T.astype(dt)),
            "cmask": cm,
            "ones": ones,
            "ident": ident,
        })
    return in_maps


def kernel(x, wq, wk, wv, wo):
    B = np.asarray(x).shape[0]
    in_maps = make_in_maps(x, wq, wk, wv, wo)
    nc = _get_nc()
    res = run_bass_kernel_spmd(nc, in_maps, core_ids=list(range(NCORES)))
    acc = res.results[0]["out"].astype(np.float32)
    for c in range(1, NCORES):
        acc = acc + res.results[c]["out"].astype(np.float32)
    return acc.reshape(B, S, HID)


if __name__ == "__main__":
    # smoke test against numpy reference
    rng = np.random.default_rng(0)
    x = rng.standard_normal((1, S, HID), dtype=np.float32)
    lim = float(np.sqrt(6.0 / (HID + 16 * HD)))
    wq, wk, wv, wo = (rng.uniform(-lim, lim, (1024, 1024)).astype(np.float32)
                      for _ in range(4))
    got = kernel(x=x, wq=wq, wk=wk, wv=wv, wo=wo)
    print("kernel output", got.shape, got.dtype, got.flat[:4])


# revision 38
# speedup vs baseline: 1.1701x; 1.1701x over previous
"""Causal multi-head attention (B=1, S=4096, H=16 heads x 64, hidden 1024) on
8 Trainium2 NeuronCores.

Sharding: tensor-parallel over heads, 2 heads per core. Each core receives the
full activation (pre-transposed to [hidden, S] layout), its 128-row slice of
wq/wk/wv (transposed) and 128-column slice of wo (transposed), computes
q/k/v projections + flash-style causal attention for its 2 heads, applies its
slice of the output projection, and writes a full-shape partial output (fp16).
The host sums the 8 partials (the TP all-reduce) to produce the final output.

Kernel layout notes (v3):
  - scores are computed TRANSPOSED: ST[sk, sq] = kT_tile^T @ qT_block, so the
    softmax numerator exp() runs PSUM->SBUF on the scalar engine with no
    transposes of the probability matrix anywhere.
  - a slot = ONE 128-row k-tile x BOTH heads. The two heads' score matmuls
    have K=64 (head dim) and disjoint stationary partition ranges (0-63 /
    64-127), so bass auto-derives row_grp=h0/h64 tile positions and the PE
    runs the pair CONCURRENTLY in the two halves of the array (row tiling).
    This halves the PE time of the score phase vs. sequential K=64 matmuls.
  - both heads share ONE exp() instruction per slot: h0's valid columns land
    at [off, 512), h64's are packed at [512, 1024-off), so the valid region
    is contiguous and one activation covers it. The scalar engine is the
    bottleneck engine (exp over the whole causal triangle is ~110us of pure
    element throughput), so no wasted exp columns and as few instruction
    access-latency taxes as possible.
  - the softmax denominator comes for free from the PV matmul by augmenting
    v with a ones column (stationary operand [v | 1], M=65): output row 64
    accumulates sum_k exp(s).
  - per-head l rows are broadcast across the head's 64 partitions with a
    pair of col-tiled (M=64) PE matmuls (concurrent), and 1/l = exp(-ln l)
    runs on the scalar engine. (The DVE reciprocal is a ~3.4us multi-pass
    instruction that stalls the FIFO vector queue and with it the whole
    block boundary — measured; Ln/Exp slot into the exp stream instead.)
  - matmul operands are float16 (11-bit mantissa ~ TF32 accuracy): 2-byte
    weight loads pipeline behind the matmul stream and keep the PE's HAM
    clock gate warm (4-byte fp32/fp32r weight loads serialize, halving
    effective clock). PSUM accumulation is fp32 throughout. Output partials
    are stored fp16 (halves the output DMA) and summed fp32 on the host.
  - emission is a software-pipelined work queue with per-piece deadlines:
    each attention slot emits the row-tiled score pair, exp, mask, and a
    4-slot-lagged PV pair, plus one drip-fed piece of deferred work (the
    previous block's normalize/out-proj, the next block's projections).
    Pieces whose deadline allows it carry across block boundaries so the
    boundary never runs a serial flush while the scalar engine idles.
    Keeping the PE dense also keeps the HAM clock gate at 8/8 (2.4 GHz).
  - weights/x use a (p k) contraction-chunk layout so every weight-load DMA
    descriptor is a contiguous 2KB line (the (k p) layout's 256B descriptors
    made the startup weight loads descriptor-bound). A burst of tiny warm-up
    matmuls on a memset tile trips the HAM activity window during the
    startup DMA wait so the first projections run at 2.4 GHz.
  - the last block's normalize/out-proj is emitted per 128-column chunk,
    pinned right after the PV that finalizes that chunk, so the kernel tail
    is a staggered pipeline instead of a serial norm -> 8 matmuls -> stores
    chain.
"""
import sys
sys.path.insert(0, "/opt/trn_rl_repo")

import numpy as np

import concourse.bass as bass
import concourse.mybir as mybir
import concourse.tile as tile
from concourse.bass_utils import run_bass_kernel_spmd

# ---------------------------------------------------------------- constants
S = 4096          # sequence length
HID = 1024        # hidden dim
NCORES = 8
HPC = 2           # heads per core
HD = 64           # head dim
EPC = HPC * HD    # 128 e-dims (head-concat) per core
SB = 512          # q-block width
NB = S // SB      # 8 q-blocks
NT = S // 128     # 32 k-tiles

F32 = mybir.dt.float32
F16 = mybir.dt.float16
DT = F16  # matmul operand dtype

_MAX_WAITS = 1    # this walrus build allows a single sync-wait per instruction


def _split_waits(nc):
    """Hoist extra sync-waits onto inserted same-engine drain carriers."""
    n = 0
    for fn in nc.m.functions:
        for bb in fn.blocks:
            insts = bb.instructions
            i = 0
            while i < len(insts):
                inst = insts[i]
                si = inst.sync_info
                w = list(si.on_wait) if si is not None and si.on_wait else []
                if len(w) > _MAX_WAITS:
                    chunks = [w[j:j + _MAX_WAITS] for j in range(0, len(w), _MAX_WAITS)]
                    si.on_wait = chunks[-1]
                    for ch in chunks[:-1]:
                        # EventSemaphore carrier: pure wait, no pipeline flush
                        # (InstDrain stalls the engine pipe ~1.5us per use).
                        d = mybir.InstEventSemaphore(
                            name=f"{inst.name}_ws{n}", ins=[], outs=[])
                        d.engine = inst.engine
                        d.sync_info = mybir.SyncInfo(on_wait=ch, on_update=[])
                        insts.insert(i, d)
                        i += 1
                        n += 1
                i += 1
    return n


def _build_nc():
    nc = bass.Bass(target_bir_lowering=False)

    xT = nc.declare_dram_parameter("xT", [HID, S], DT, isOutput=False)
    wqT = nc.declare_dram_parameter("wqT", [HID, EPC], DT, isOutput=False)
    wkT = nc.declare_dram_parameter("wkT", [HID, EPC], DT, isOutput=False)
    wvT = nc.declare_dram_parameter("wvT", [HID, EPC], DT, isOutput=False)
    woT = nc.declare_dram_parameter("woT", [EPC, HID], DT, isOutput=False)
    cmask = nc.declare_dram_parameter("cmask", [128, 128], DT, isOutput=False)
    ones = nc.declare_dram_parameter("ones", [1, 128], DT, isOutput=False)
    ident = nc.declare_dram_parameter("ident", [128, 128], DT, isOutput=False)
    out = nc.declare_dram_parameter("out", [S, HID], DT, isOutput=True)

    KH = HID // 128  # 8 contraction chunks for projections

    with tile.TileContext(nc) as tc:
        with tc.tile_pool(name="const", bufs=1) as const, \
             tc.tile_pool(name="qk", bufs=1) as qk, \
             tc.tile_pool(name="xt", bufs=3) as xtp, \
             tc.tile_pool(name="vt", bufs=2) as vtp, \
             tc.tile_pool(name="pt", bufs=5) as ptp, \
             tc.tile_pool(name="att", bufs=2) as attp, \
             tc.tile_pool(name="osb", bufs=4) as osbp, \
             tc.tile_pool(name="rl", bufs=4) as rlp, \
             tc.tile_pool(name="bc", bufs=2) as bcp, \
             tc.tile_pool(name="st", bufs=2, space="PSUM") as stp, \
             tc.tile_pool(name="sc", bufs=2, space="PSUM") as scp, \
             tc.tile_pool(name="ot", bufs=2, space="PSUM") as otp:

            wq_sb = const.tile([128, KH, EPC], DT, tag="wq")
            wk_sb = const.tile([128, KH, EPC], DT, tag="wk")
            wv_sb = const.tile([128, KH, EPC], DT, tag="wv")
            wo_sb = const.tile([EPC, HID], DT, tag="wo")
            cm_sb = const.tile([128, 128], DT, tag="cm")
            ones_sb = const.tile([1, 128], DT, tag="ones")
            id_sb = const.tile([128, 128], DT, tag="id")
            wtile = const.tile([128, 512], DT, tag="warm")
            wdump = const.tile([64, 1], DT, tag="wdump")
            qT = qk.tile([128, S], DT, tag="qT")   # [e(2 heads), s]
            kT = qk.tile([128, S], DT, tag="kT")
            vbuf = qk.tile([128, HPC, NT, 65], DT, tag="v")  # [sk, h, t, v|1]

            def warmup_mms():
                # trip the HAM activity window during the startup DMA wait so
                # the first projections run at 2.4 GHz instead of 1.2
                nc.gpsimd.memset(wtile, 0.25)
                wps = scp.tile([128, 512], F32, tag="sc", name="warmps")
                # just enough to bridge to the first projections (which then
                # keep HAM busy themselves) — a longer burst delays them
                for _ in range(6):
                    nc.tensor.matmul(wps[0:64, :], wtile[:, 0:64],
                                     wtile, start=True, stop=True)
                return wps

            def load_consts():
                # weights spread across independent DMA queues; (p k) layout
                # = one contiguous 2KB descriptor per partition
                # all weights on the scalar HWDGE queue in need-order, only
                # tiny constants on gpsimd: active queues share the 16 DMA
                # engines, so every extra busy queue cuts the critical path's
                # share (sync carries xt only)
                nc.scalar.dma_start(
                    out=wq_sb, in_=wqT.rearrange("(p k) m -> p k m", p=128))
                nc.scalar.dma_start(
                    out=wk_sb, in_=wkT.rearrange("(p k) m -> p k m", p=128))
                nc.scalar.dma_start(
                    out=wv_sb, in_=wvT.rearrange("(p k) m -> p k m", p=128))
                nc.scalar.dma_start(out=wo_sb, in_=woT[:, :])
                nc.gpsimd.dma_start(out=id_sb, in_=ident[:, :])
                nc.gpsimd.dma_start(out=cm_sb, in_=cmask[:, :])
                nc.gpsimd.dma_start(out=ones_sb, in_=ones[:, :])
                nc.vector.memset(
                    vbuf.rearrange("p a b c -> p (a b c)"), 1.0)

            def load_xt(bb):
                xt = xtp.tile([128, KH, SB], DT, tag="xt", name=f"xt{bb}")
                src_ap = xT.rearrange("(p k) s -> p k s", p=128)
                if bb == 0:
                    # split the first load so the first projection matmuls
                    # (k-chunks 0..3) can start after half the transfer
                    nc.sync.dma_start(out=xt[:, 0:KH // 2, :],
                                      in_=src_ap[:, 0:KH // 2, 0:SB])
                    nc.sync.dma_start(out=xt[:, KH // 2:, :],
                                      in_=src_ap[:, KH // 2:, 0:SB])
                else:
                    nc.sync.dma_start(
                        out=xt, in_=src_ap[:, :, bb * SB:(bb + 1) * SB])
                return xt

            def norm_chunk(b, ots, m, csl):
                """Normalize att columns [m*128,(m+1)*128) of block b.
                csl = slice(m*128, (m+1)*128). Allocations are lazy (inside
                the closure) so the scratch-pool rotation follows run order."""
                holder = nrm_state[b]
                if m == 0:
                    holder["att"] = attp.tile([128, SB], DT, tag="att",
                                              name=f"att{b}")
                att = holder["att"]
                w = csl.stop - csl.start
                bcps = scp.tile([128, 512], F32, tag="sc", name=f"bc{b}_{m}")
                t1 = bcp.tile([128, 512], F32, tag="bc", name=f"ln{b}_{m}")
                lrows = []
                for h in range(HPC):
                    lrow = rlp.tile([1, SB], DT, tag="rl", name=f"rl{b}_{h}_{m}")
                    nc.vector.tensor_copy(out=lrow[:, 0:w],
                                          in_=ots[h][64:65, csl])
                    lrows.append(lrow)
                # col-tiled pair: out base partitions 0/64 -> concurrent
                for h in range(HPC):
                    nc.tensor.matmul(bcps[64 * h:64 * (h + 1), 0:w],
                                     ones_sb[:, 0:64], lrows[h][:, 0:w],
                                     start=True, stop=True)
                # 1/l = exp(-ln l) on the scalar engine
                nc.scalar.activation(out=t1[:, 0:w], in_=bcps[:, 0:w],
                                     func=mybir.ActivationFunctionType.Ln)
                nc.scalar.activation(out=t1[:, 0:w], in_=t1[:, 0:w],
                                     func=mybir.ActivationFunctionType.Exp,
                                     scale=-1.0)
                for h in range(HPC):
                    nc.vector.tensor_mul(att[64 * h:64 * (h + 1), csl],
                                         ots[h][0:64, csl],
                                         t1[64 * h:64 * (h + 1), 0:w])

            def outproj(b, mlist, tail=False):
                att = nrm_state[b]["att"]
                for m in mlist:
                    osb = osbp.tile([128, HID], DT, tag="osb",
                                    name=f"osb{b}_{m}")
                    for n2 in range(2):
                        op = scp.tile([128, 512], F32, tag="sc",
                                      name=f"op{b}_{m}_{n2}")
                        nc.tensor.matmul(op, att[:, m * 128:(m + 1) * 128],
                                         wo_sb[:, n2 * 512:(n2 + 1) * 512],
                                         start=True, stop=True)
                        if tail and n2 == 1:
                            # at the kernel tail the scalar engine is idle:
                            # split the evictions across DVE and scalar so
                            # the PSUM->SBUF chain isn't DVE-serial
                            nc.scalar.activation(
                                out=osb[:, n2 * 512:(n2 + 1) * 512], in_=op,
                                func=mybir.ActivationFunctionType.Copy)
                        else:
                            nc.vector.tensor_copy(
                                out=osb[:, n2 * 512:(n2 + 1) * 512], in_=op)
                    r0 = (4 * b + m) * 128
                    # stores ride the (otherwise idle) GpSimd SWDGE queues
                    nc.gpsimd.dma_start(out=out[r0:r0 + 128, :], in_=osb)

            def make_norm_pieces(b, ots):
                """Block b's normalize+out-proj as drip pieces: [bcast+recip
                (whole block), op0, op1, op2, op3]."""
                def norm_all():
                    norm_chunk(b, ots, 0, slice(0, SB))
                return [norm_all] + \
                    [lambda m=m: outproj(b, [m]) for m in range(4)]

            def make_proj_chunks(bb):
                """Projection work for block bb as slot-sized pieces (each
                ~2-4 matmuls of PE work, so a drip-fed piece never delays the
                next slot's score matmuls for long)."""
                slb = slice(bb * SB, (bb + 1) * SB)
                holder = {}

                def c_w(w_sb, dst, half):
                    def run():
                        if half == 0:
                            ps = scp.tile([128, SB], F32, tag="sc",
                                          name=f"ps{bb}_{dst.name}")
                            holder[dst.name] = ps
                        else:
                            ps = holder[dst.name]
                        for k in range(KH // 2 * half, KH // 2 * (half + 1)):
                            nc.tensor.matmul(ps, w_sb[:, k, :], xts[bb][:, k, :],
                                             start=(k == 0), stop=(k == KH - 1))
                        if half == 1:
                            nc.vector.tensor_copy(out=dst[:, slb], in_=ps)
                    return run

                def c_v(half):
                    def run():
                        if half == 0:
                            ps = scp.tile([128, SB], F32, tag="sc",
                                          name=f"psv{bb}")
                            holder["psv"] = ps
                        else:
                            ps = holder["psv"]
                        for k in range(KH // 2 * half, KH // 2 * (half + 1)):
                            nc.tensor.matmul(ps, wv_sb[:, k, :], xts[bb][:, k, :],
                                             start=(k == 0), stop=(k == KH - 1))
                        if half == 1:
                            vt = vtp.tile([128, SB], DT, tag="vt",
                                          name=f"vt{bb}")
                            nc.vector.tensor_copy(out=vt, in_=ps)
                            holder["vt"] = vt
                    return run

                def c_flips(jj):
                    def run():
                        vt = holder["vt"]
                        for j in jj:
                            t = 4 * bb + j
                            fp = scp.tile([128, 128], F32, tag="sc",
                                          name=f"fp{bb}_{j}")
                            nc.tensor.matmul(fp, vt[:, j * 128:(j + 1) * 128],
                                             id_sb, start=True, stop=True)
                            nc.vector.tensor_copy(out=vbuf[:, 0, t, 0:64],
                                                  in_=fp[:, 0:64])
                            nc.vector.tensor_copy(out=vbuf[:, 1, t, 0:64],
                                                  in_=fp[:, 64:128])
                    return run

                return [c_w(wq_sb, qT, 0), c_w(wq_sb, qT, 1),
                        c_w(wk_sb, kT, 0), c_w(wk_sb, kT, 1),
                        c_v(0), c_v(1),
                        c_flips([0, 1]), c_flips([2, 3])]

            xts = {}
            nrm_state = {b: {} for b in range(NB)}

            wps = warmup_mms()
            xts[0] = load_xt(0)      # xt(0) DMA first on the sync queue
            load_consts()            # (vbuf memset heads the DVE queue)
            xts[1] = load_xt(1)      # behind xt(0), ready by block 0's drip
            # consume the warm-up tile so the scratch pool sees it freed
            nc.vector.tensor_copy(out=wdump, in_=wps[0:64, 0:1])
            for c in make_proj_chunks(0):   # bootstrap block 0 projections
                c()

            pending = []    # (deadline_block, closure) work queue
            prev_no = []    # norm/out-proj pieces of the previous block
            for b in range(NB):
                pjn = make_proj_chunks(b + 1) if b + 1 < NB else None
                no = prev_no
                # queue: norm first (PV of this block waits on the previous
                # block's ot buffers), then xt prefetch + q-projection (hard
                # deadline: next block's first slot), then out-projections
                # interleaved between COMPLETE projection half-pairs (the
                # 2-buffer PSUM scratch rotation requires a tile's eviction
                # emitted before the tile allocated 2 later is written).
                adds = []
                if no:
                    adds.append((b, no[0]))          # bcast + recip + muls
                if b + 2 < NB:
                    def mk_pref(bb=b + 2):
                        def run():
                            xts[bb] = load_xt(bb)
                        return run
                    adds.append((b, mk_pref()))      # xt prefetch (2 ahead)
                head = adds
                mid = []
                if pjn:
                    mid += [(b, pjn[0]), (b, pjn[1])]    # q-proj halves
                inter = []
                ops = list(no[1:]) if no else []
                pairs = [pjn[2:4], pjn[4:6], pjn[6:8]] if pjn else [[], [], []]
                if ops:
                    inter.append((b + 1, ops.pop(0)))
                for pr in pairs:
                    inter += [(b + 1, c) for c in pr]
                    if ops:
                        inter.append((b + 1, ops.pop(0)))
                inter += [(b + 1, c) for c in ops]
                pending = head + pending + mid + inter

                # ---------- attention for q-block b (both heads per slot)
                ntl = 4 * (b + 1)  # causal k-tiles
                ots = [otp.tile([65, SB], F32, tag="ot", name=f"ot{b}_{h}")
                       for h in range(HPC)]
                pend = []   # pending (t, off, pt) awaiting PV emission
                tail_sp = (None, 0)   # last diag st tile's spare columns

                def emit_pv(t, off, pt):
                    # h0 valid pt cols [off, 512); h64 packed [512, 1024-off)
                    nc.tensor.matmul(
                        ots[0][:, off:], vbuf[:, 0, t, :], pt[:, off:SB],
                        start=(t == 0), stop=(t == ntl - 1),
                        skip_group_check=True)
                    nc.tensor.matmul(
                        ots[1][:, off:], vbuf[:, 1, t, :],
                        pt[:, SB:2 * SB - off],
                        start=(t == 0), stop=(t == ntl - 1),
                        skip_group_check=True)

                last = b == NB - 1
                # pace the deferred-piece drip over slots [0, ntl-4): the
                # last slots stay drip-free so the PE carries no backlog
                # into the next block's score matmuls (measured: a late
                # dripped projection delays them ~3us and the scalar engine
                # idles the whole time)
                Q = len(pending)
                W = max(1, ntl - 4)
                emitted = 0
                for t in range(ntl):
                    off = 128 * (t - 4 * b) if t >= 4 * b else 0
                    st = stp.tile([128, 2 * SB], F32, tag="st",
                                  name=f"st{b}_{t}")
                    # the two heads' K=64 score matmuls: disjoint stationary
                    # partition ranges -> row-tiled, run concurrently
                    nc.tensor.matmul(
                        st[:, off:SB],
                        kT[0:64, t * 128:(t + 1) * 128],
                        qT[0:64, b * SB + off:(b + 1) * SB],
                        start=True, stop=True)
                    nc.tensor.matmul(
                        st[:, SB:2 * SB - off],
                        kT[64:128, t * 128:(t + 1) * 128],
                        qT[64:128, b * SB + off:(b + 1) * SB],
                        start=True, stop=True)
                    if last and off:
                        # the last block's diagonal slots are small; keep
                        # the HAM activity window busy with a warm-up matmul
                        # into the st tile's unused spare columns, else the
                        # whole tail (PVs, out-proj, norms) runs at 1.2 GHz
                        tail_sp = (st, off)
                        for _ in range(3):
                            nc.tensor.matmul(st[0:64, 2 * SB - off:],
                                             wtile[:, 0:64], wtile[:, 0:off],
                                             start=True, stop=True)
                    pt = ptp.tile([128, 2 * SB], DT, tag="pt",
                                  name=f"pt{b}_{t}")
                    nc.scalar.activation(out=pt[:, off:2 * SB - off],
                                         in_=st[:, off:2 * SB - off],
                                         func=mybir.ActivationFunctionType.Exp,
                                         scale=float(HD) ** -0.5)
                    if t >= 4 * b:  # triangle mask on the 128-wide diagonal
                        nc.vector.tensor_mul(pt[:, off:off + 128],
                                             pt[:, off:off + 128],
                                             cm_sb[:, 0:128])
                        nc.vector.tensor_mul(pt[:, SB:SB + 128],
                                             pt[:, SB:SB + 128],
                                             cm_sb[:, 0:128])
                    pend.append((t, off, pt))
                    # drip BEFORE the PV pops: a dripped piece may produce
                    # the vbuf tiles the popped PV consumes (same-slot case),
                    # and the PE queue is FIFO — the producer must be emitted
                    # first or the queue deadlocks
                    if t < W:
                        target = ((t + 1) * Q + W - 1) // W
                        while emitted < min(target, emitted + 2) and pending:
                            pending.pop(0)[1]()
                            emitted += 1
                    # PV lags 4 slots (hides the block-start ot/norm chain),
                    # shrinking near the block end so ot columns finalize
                    # early enough to pipeline the tail
                    lag = 4 if t < ntl - 7 else (2 if t < ntl - 3 else 1)
                    while len(pend) > lag:
                        emit_pv(*pend.pop(0))
                    if last and t >= ntl - 3:
                        # tail pipeline: chunk m's ot columns are final once
                        # PV(ntl-4+m) has run (popped at slot ntl-3+m).
                        # Out-projections wait until every PV is emitted —
                        # an out-proj matmul ahead of a PV in the PE FIFO
                        # serializes the whole tail (measured 4.3us/chunk).
                        m = t - (ntl - 3)
                        norm_chunk(b, ots, m, slice(m * 128, (m + 1) * 128))
                        if tail_sp[0] is not None:
                            tst, toff = tail_sp
                            nc.tensor.matmul(tst[0:64, 2 * SB - toff:],
                                             wtile[:, 0:64],
                                             wtile[:, 0:toff],
                                             start=True, stop=True)
                def tail_warm(n):
                    if last and tail_sp[0] is not None:
                        tst, toff = tail_sp
                        for _ in range(n):
                            nc.tensor.matmul(tst[0:64, 2 * SB - toff:],
                                             wtile[:, 0:64], wtile[:, 0:toff],
                                             start=True, stop=True)

                while pend:
                    t0 = pend[0][0]
                    emit_pv(*pend.pop(0))
                    tail_warm(4)
                    if last and t0 >= ntl - 4:
                        m = t0 - (ntl - 4)
                        norm_chunk(b, ots, m, slice(m * 128, (m + 1) * 128))
                if last:
                    for m in range(4):
                        outproj(b, [m], tail=True)
                        tail_warm(4)
                # flush pieces whose deadline is this block; carry the rest
                flush = [c for dl, c in pending if dl <= b]
                pending = [(dl, c) for dl, c in pending if dl > b]
                for c in flush:
                    c()
                prev_no = make_norm_pieces(b, ots) if not last else []

            for dl, c in pending:
                c()

    _split_waits(nc)
    return nc


_cached = {}


def _get_nc():
    if "nc" not in _cached:
        _cached["nc"] = _build_nc()
    return _cached["nc"]


def make_in_maps(x, wq, wk, wv, wo):
    x = np.asarray(x, dtype=np.float32)
    wq, wk, wv, wo = (np.asarray(a, dtype=np.float32) for a in (wq, wk, wv, wo))
    B = x.shape[0]
    assert x.shape == (B, S, HID)

    dt = np.float16
    xT = np.ascontiguousarray(x[0].T.astype(dt))            # [HID, S]
    # static causal mask for the 128-wide diagonal blocks
    p = np.arange(128)[:, None]
    i = np.arange(128)[None, :]
    cm = (p <= i).astype(dt)                                # [128, 128]
    ones = np.ones((1, 128), dtype=dt)
    ident = np.eye(128, dtype=dt)

    in_maps = []
    for c in range(NCORES):
        esl = slice(c * EPC, (c + 1) * EPC)
        in_maps.append({
            "xT": xT,
            "wqT": np.ascontiguousarray(wq[esl, :].T.astype(dt)),
            "wkT": np.ascontiguousarray(wk[esl, :].T.astype(dt)),
            "wvT": np.ascontiguousarray(wv[esl, :].T.astype(dt)),
            "woT": np.ascontiguousarray(wo[:, esl].T.astype(dt)),
            "cmask": cm,
            "ones": ones,
            "ident": ident,
        })
    return in_maps


def kernel(x, wq, wk, wv, wo):
    B = np.asarray(x).shape[0]
    in_maps = make_in_maps(x, wq, wk, wv, wo)
    nc = _get_nc()
    res = run_bass_kernel_spmd(nc, in_maps, core_ids=list(range(NCORES)))
    acc = res.results[0]["out"].astype(np.float32)
    for c in range(1, NCORES):
        acc = acc + res.results[c]["out"].astype(np.float32)
    return acc.reshape(B, S, HID)


if __name__ == "__main__":
    # smoke test against numpy reference
    rng = np.random.default_rng(0)
    x = rng.standard_normal((1, S, HID), dtype=np.float32)
    lim = float(np.sqrt(6.0 / (HID + 16 * HD)))
    wq, wk, wv, wo = (rng.uniform(-lim, lim, (1024, 1024)).astype(np.float32)
                      for _ in range(4))
    got = kernel(x=x, wq=wq, wk=wk, wv=wv, wo=wo)
    print("kernel output", got.shape, got.dtype, got.flat[:4])
